# revision 11
# baseline (speedup 1.0000x reference)
"""Trainium2 Bass kernel for nn_ConstrainedEnhancementModel.

Contract: kernel(**inputs) takes the FULL unsharded inputs (as produced by
reference.setup_inputs()) and returns the FULL [4096, 2000, 6] float32 output.

Strategy (pure data parallel over 8 NeuronCores, 512 batch rows each):
  - Feature-major MLP chain in fp8 (e4m3) with DoubleRow matmuls: weights are
    scaled x64 into fp8's normal range, activations apply scale=1/64 on the
    psum read so stored activations stay raw-scale fp8.
  - x arrives host-side pre-transposed into the window-blocked layout
    (partition 32w+r = x col 24*(4*i4+w)+r, free = i4*512 + batch), in both
    fp8 (for L1) and bf16 (for the interpolation matmul).  The label
    embedding rows emb[labels].T are also selected host-side.
  - Final layer: out = h5 @ (W6 * c_d * 256) + x @ (G * 256), evaluated per
    output window (480 cols); fp8 DoubleRow pairs for the W6 part, a K=32
    bf16 matmul on a 32-row PE tile for the G (lin-interp + b6) part.  The
    psum->sbuf copy applies 1/256 and writes bf16; output DMAs one
    [128, nwin*480] chunk per (group, batch-tile).
  - Output tensor is bf16 (within the rel-err budget); host upcasts to f32.
"""

import numpy as np
import ml_dtypes

import bass_rust
import concourse.bass as bass
import concourse.bacc as bacc
import concourse.mybir as mybir
import concourse.tile as tile
from concourse import bass_utils

F32 = mybir.dt.float32
BF16 = mybir.dt.bfloat16
F8 = mybir.dt.float8e4
BF16_NP = ml_dtypes.bfloat16
F8_NP = ml_dtypes.float8_e4m3fn

# Problem config (hardcoded; must match the reference)
LOW_T = 100
HIGH_T = 2000
FEAT = 6
HID = 256
NUM_CLASSES = 10
LBL_DIM = 16
UP = 20
B = 4096
NCORES = 8
BC = B // NCORES          # 512 batch rows per core
NBT = BC // 128           # 4 batch tiles per core
D_IN = LOW_T * FEAT       # 600
D_OUT = HIGH_T * FEAT     # 12000
NW = 25                   # output windows (80 timesteps * 6 feats = 480 cols)
WT = 480
NI4 = 7                   # ceil(25/4) groups of 4 windows
EW = 64.0                 # encoder weight fp8 scale
SC = 256.0                # decoder/W6/G fp8+psum scale
DR = mybir.MatmulPerfMode.DoubleRow
I4_ORDER = [0, 1, 2, 3, 4, 5, 6]   # small group last: minimal tail

# wenc blob column offsets (fp8, ktile-major within each layer)
OW1 = 0            # 7 ktiles x 512
OW2 = 3584         # 4 ktiles x 256
OW3 = 4608         # 2 ktiles x 128
OW4 = 4864         # 2 ktiles x 256
OW5 = 5376         # 2 ktiles x 512
WENC = 6400
OCG = NI4 * 512    # cg offset inside the xbg blob


def _ap3(t, col_off, stride2, n3):
    """3-dim AP over all 128 partitions of tile t: [128, 2, n3]."""
    a = t[:]
    return bass_rust.AP(
        tensor=a.tensor, offset=a.offset + col_off,
        ap=[[a.ap[0][0], 128], [stride2, 2], [1, n3]],
    )


def _build_nc():
    """Build the single-core Bass program (SPMD: same program on all 8)."""
    nc = bacc.Bacc("TRN2", target_bir_lowering=False, debug=False)

    wenc_d = nc.dram_tensor("wenc", [128, WENC], F8, kind="ExternalInput")
    x8_d = nc.dram_tensor("x8", [128, NI4 * 512], F8, kind="ExternalInput")
    bias_d = nc.dram_tensor("biasb", [128, 26], F32, kind="ExternalInput")
    emb_d = nc.dram_tensor("l4emb", [32, BC], F8, kind="ExternalInput")
    xbg_d = nc.dram_tensor("xbg", [128, NI4 * 512 + NI4 * WT], BF16,
                           kind="ExternalInput")
    w6_d = nc.dram_tensor("w6p", [128, NW * 4 * WT], F8, kind="ExternalInput")
    y_d = nc.dram_tensor("y", [BC, D_OUT], BF16, kind="ExternalOutput")

    RELU = mybir.ActivationFunctionType.Relu
    IDENT = mybir.ActivationFunctionType.Identity
    ADD = mybir.AluOpType.add
    MAX = mybir.AluOpType.max
    MULT = mybir.AluOpType.mult

    with tile.TileContext(nc) as tc:
        with (
            tc.tile_pool(name="const", bufs=1) as cp,
            tc.tile_pool(name="vtpool", bufs=2) as vp,
            tc.tile_pool(name="outpool", bufs=4) as op,
            tc.tile_pool(name="ppool", bufs=8, space="PSUM") as pm,
        ):
            # ---- persistent SBUF tensors ----
            wenc = cp.tile([128, WENC], F8, tag="wenc", name="wenc")
            x8 = cp.tile([128, NI4 * 512], F8, tag="x8", name="x8")
            cbias = cp.tile([128, 26], F32, tag="cbias", name="cbias")
            xbg = cp.tile([128, NI4 * 512 + NI4 * WT], BF16, tag="xbg", name="xbg")
            w6all = cp.tile([128, NW * 4 * WT], F8, tag="w6all", name="w6all")
            h1 = cp.tile([128, 4 * BC], F8, tag="h1", name="h1")
            h2 = cp.tile([128, 2 * BC], F8, tag="h2", name="h2")
            l4r = cp.tile([128, 2 * BC], F8, tag="l4r", name="l4r")
            h4 = cp.tile([128, 2 * BC], F8, tag="h4", name="h4")
            h5 = cp.tile([128, 4 * BC], F8, tag="h5", name="h5")

            # l4r ktile1 rows 32-127 must be zero (rows 0-31 come via DMA)
            for p0 in (32, 64, 96):
                nc.gpsimd.memset(l4r[p0:p0 + 32, BC:2 * BC], 0.0)

            # ---- loads, ordered by first use ----
            nc.sync.dma_start(x8[:], x8_d[:])
            nc.sync.dma_start(wenc[:, 0:OW2], wenc_d[:, 0:OW2])
            nc.sync.dma_start(cbias[:], bias_d[:])
            nc.sync.dma_start(l4r[0:32, BC:2 * BC], emb_d[:])
            nc.sync.dma_start(wenc[:, OW2:WENC], wenc_d[:, OW2:WENC])
            nc.sync.dma_start(xbg[:], xbg_d[:])
            for g in I4_ORDER:
                nwin = 4 if g < 6 else 1
                o = g * 4 * WT * 4
                nc.sync.dma_start(
                    w6all[:, o:o + nwin * 4 * WT], w6_d[:, o:o + nwin * 4 * WT]
                )

            # bias column views: 0-12 raw (scalar engine), 13-25 x64 (vector)
            cb1 = [cbias[:, m:m + 1] for m in range(4)]
            cb2 = [cbias[:, 4 + m:5 + m] for m in range(2)]
            cb4 = [cbias[:, 7 + m:8 + m] for m in range(2)]
            cb5 = [cbias[:, 9 + m:10 + m] for m in range(4)]
            vb1 = [cbias[:, 13 + m:14 + m] for m in range(4)]
            vb2 = [cbias[:, 17 + m:18 + m] for m in range(2)]
            vb3 = cbias[:, 19:20]
            vb4 = [cbias[:, 20 + m:21 + m] for m in range(2)]
            vb5 = [cbias[:, 22 + m:23 + m] for m in range(4)]

            def act_scalar(dst, ps, cb, func=RELU):
                nc.scalar.activation(dst, ps, func, bias=cb, scale=1.0 / EW)

            def act_vector(dst, ps, vb):
                # (psum + 64b) then (max 0)*(1/64); two DVE ops
                vt = vp.tile([128, BC], F32, tag="vt", name="vt")
                nc.vector.tensor_scalar(vt[:], ps, vb, None, ADD)
                nc.vector.tensor_scalar(dst, vt[:], 0.0, 1.0 / EW, MAX, MULT)

            # ---- PE warm-up: keep HAM clocked up while input DMAs land ----
            dmy = cp.tile([128, 128], F8, tag="dmy", name="dmy")
            nc.gpsimd.memset(dmy[:], 0.0)
            for _ in range(16):
                psd = pm.tile([128, 512], F32, tag="ps", name="ps")
                nc.tensor.matmul(psd[:, 0:128], dmy[:], dmy[:],
                                 start=True, stop=True)

            # ---- encoder MLP (feature-major, fp8 DoubleRow) ----
            # L1: [600->512] window-blocked x, 7 ktiles = 3 DR pairs + 1 plain
            for m in range(4):
                ps = pm.tile([128, 512], F32, tag="ps", name="ps")
                for p in range(3):
                    nc.tensor.matmul(
                        ps[:, 0:BC],
                        _ap3(wenc, OW1 + 2 * p * 512 + m * 128, 512, 128),
                        _ap3(x8, 2 * p * 512, 512, 512),
                        start=(p == 0), stop=False, perf_mode=DR,
                    )
                nc.tensor.matmul(
                    ps[:, 0:BC],
                    wenc[:, OW1 + 6 * 512 + m * 128:OW1 + 6 * 512 + (m + 1) * 128],
                    x8[:, 6 * 512:7 * 512], start=False, stop=True,
                )
                act_scalar(h1[:, m * BC:(m + 1) * BC], ps[:, 0:BC], cb1[m])
            # L2: [512->256], 4 ktiles = 2 DR pairs
            for m in range(2):
                ps = pm.tile([128, 512], F32, tag="ps", name="ps")
                for p in range(2):
                    nc.tensor.matmul(
                        ps[:, 0:BC],
                        _ap3(wenc, OW2 + 2 * p * 256 + m * 128, 256, 128),
                        _ap3(h1, 2 * p * BC, BC, 512),
                        start=(p == 0), stop=(p == 1), perf_mode=DR,
                    )
                act_scalar(h2[:, m * BC:(m + 1) * BC], ps[:, 0:BC], cb2[m])
            # L3: [256->128] no relu -> l4r ktile0 (single vector op)
            ps = pm.tile([128, 512], F32, tag="ps", name="ps")
            nc.tensor.matmul(
                ps[:, 0:BC], _ap3(wenc, OW3, 128, 128), _ap3(h2, 0, BC, 512),
                start=True, stop=True, perf_mode=DR,
            )
            nc.vector.tensor_scalar(l4r[:, 0:BC], ps[:, 0:BC], 1.0 / EW, vb3,
                                    MULT, ADD)
            # L4: [144->256] (feat ktile + padded label ktile)
            for m in range(2):
                ps = pm.tile([128, 512], F32, tag="ps", name="ps")
                nc.tensor.matmul(
                    ps[:, 0:BC], _ap3(wenc, OW4 + m * 128, 256, 128),
                    _ap3(l4r, 0, BC, 512),
                    start=True, stop=True, perf_mode=DR,
                )
                act_scalar(h4[:, m * BC:(m + 1) * BC], ps[:, 0:BC], cb4[m])
            # L5: [256->512]
            for m in range(4):
                ps = pm.tile([128, 512], F32, tag="ps", name="ps")
                nc.tensor.matmul(
                    ps[:, 0:BC], _ap3(wenc, OW5 + m * 128, 512, 128),
                    _ap3(h4, 0, BC, 512),
                    start=True, stop=True, perf_mode=DR,
                )
                act_scalar(h5[:, m * BC:(m + 1) * BC], ps[:, 0:BC], cb5[m])

            # ---- final layer + fused constraint epilogue ----
            for i4 in I4_ORDER:
                nwin = 4 if i4 < 6 else 1
                for bt in range(NBT):
                    pss = [pm.tile([128, 512], F32, tag="ps", name="ps")[:, 0:WT]
                           for w in range(nwin)]
                    for k2 in range(2):
                        for w in range(nwin):
                            nc.tensor.matmul(
                                pss[w],
                                _ap3(h5, 2 * k2 * BC + bt * 128, BC, 128),
                                _ap3(w6all, (i4 * 4 + w) * 4 * WT + k2 * 2 * WT, WT, WT),
                                start=(k2 == 0), stop=False, perf_mode=DR,
                            )
                    for w in range(nwin):
                        p0 = 32 * w
                        nc.tensor.matmul(
                            pss[w],
                            xbg[p0:p0 + 32, i4 * 512 + bt * 128:i4 * 512 + bt * 128 + 128],
                            xbg[p0:p0 + 32, OCG + i4 * WT:OCG + (i4 + 1) * WT],
                            start=False, stop=True, tile_position=(p0, 0),
                        )
                    ob = op.tile([128, nwin * WT], BF16, tag=f"ob{nwin}", name=f"ob{nwin}")
                    for w in range(nwin):
                        if w % 2 == 0:
                            nc.scalar.mul(ob[:, w * WT:(w + 1) * WT], pss[w], 1.0 / SC)
                        else:
                            nc.vector.tensor_scalar_mul(
                                ob[:, w * WT:(w + 1) * WT], pss[w], 1.0 / SC)
                    nc.sync.dma_start(
                        y_d[bt * 128:(bt + 1) * 128,
                            i4 * 4 * WT:i4 * 4 * WT + nwin * WT],
                        ob[:],
                    )

    nc.compile()
    return nc


def _host_prep(inputs):
    """Build per-core in_maps from the full inputs."""
    x_full = np.asarray(inputs["low_res_data"], np.float32).reshape(B, D_IN)
    labels = np.asarray(inputs["labels"]).astype(np.int64)
    emb = np.asarray(inputs["emb"], np.float32)
    W6 = np.asarray(inputs["W6"], np.float32)
    b6 = np.asarray(inputs["b6"], np.float32)

    # per-timestep blend coefficients (match the reference formulas)
    t = np.arange(HIGH_T)
    seg = np.clip(t // UP, 0, LOW_T - 2)
    alpha = ((t - seg * UP) / UP).astype(np.float64)
    is_anchor = (t % UP) == 0
    interior = t < (LOW_T - 1) * UP
    blendf = np.where(is_anchor, 1.0, np.where(interior, 0.8, 0.0))
    c_d = np.where(is_anchor, 0.0, np.where(interior, 0.2, 1.0))
    c_start = blendf * (1.0 - alpha) * SC
    c_end = blendf * alpha * SC

    # G matrix, window-blocked: [128, NI4*480]; window i at partition
    # offset 32*(i%4), col block i//4.  Rows r=0..29 <-> x col 24*i + r,
    # row 30 = bias row (pairs with the 1.0 row of the x layout).
    gmat = np.zeros((128, NI4 * WT), np.float64)
    for tt in range(HIGH_T):
        i, dt = divmod(tt, 80)
        i4, wpos = divmod(i, 4)
        p0 = 32 * wpos
        sl = seg[tt] - 4 * i
        for f in range(FEAT):
            col = i4 * WT + FEAT * dt + f
            gmat[p0 + FEAT * sl + f, col] += c_start[tt]
            gmat[p0 + FEAT * (sl + 1) + f, col] += c_end[tt]
            gmat[p0 + 30, col] = c_d[tt] * SC * np.float64(b6[FEAT * tt + f])
    gmat = gmat.astype(np.float32).astype(BF16_NP)

    # W6 blob: [128, 100*480] fp8; window i block at col (i4*4+w)*1920,
    # sub-blocks [k2][ko] of 480 cols = W6 ktile (2*k2+ko) for that window.
    c_d_full = np.repeat(c_d, FEAT).astype(np.float32)
    w6s = (W6 * (c_d_full * SC)[None, :]).astype(np.float32)
    w6r = w6s.reshape(4, 128, NW, WT)
    w6blob = np.zeros((128, NW * 4 * WT), np.float32)
    for i in range(NW):
        i4, w = divmod(i, 4)
        for kt in range(4):
            o = (i4 * 4 + w) * 4 * WT + kt * WT
            w6blob[:, o:o + WT] = w6r[kt, :, i, :]
    w6blob = w6blob.astype(F8_NP)

    # encoder weight blob (fp8, x64, ktile-major)
    wenc = np.zeros((128, WENC), np.float32)
    W1 = np.asarray(inputs["W1"], np.float32)
    for c in range(D_IN):
        i, r = divmod(c, 24)
        i4, wpos = divmod(i, 4)
        wenc[32 * wpos + r, OW1 + i4 * 512:OW1 + (i4 + 1) * 512] = W1[c, :] * EW
    W2 = np.asarray(inputs["W2"], np.float32) * EW
    for kt in range(4):
        wenc[:, OW2 + kt * 256:OW2 + (kt + 1) * 256] = W2[kt * 128:(kt + 1) * 128]
    W3 = np.asarray(inputs["W3"], np.float32) * EW
    for kt in range(2):
        wenc[:, OW3 + kt * 128:OW3 + (kt + 1) * 128] = W3[kt * 128:(kt + 1) * 128]
    W4 = np.asarray(inputs["W4"], np.float32) * EW
    wenc[:, OW4:OW4 + 256] = W4[:128]
    wenc[0:16, OW4 + 256:OW4 + 512] = W4[128:144]
    W5 = np.asarray(inputs["W5"], np.float32) * EW
    for kt in range(2):
        wenc[:, OW5 + kt * 512:OW5 + (kt + 1) * 512] = W5[kt * 128:(kt + 1) * 128]
    wenc = wenc.astype(F8_NP)

    # bias blob [128, 26] f32: cols 0-12 raw, 13-25 pre-scaled x64
    biasb = np.zeros((128, 26), np.float32)
    biasb[:, 0:4] = np.asarray(inputs["b1"], np.float32).reshape(4, 128).T
    biasb[:, 4:6] = np.asarray(inputs["b2"], np.float32).reshape(2, 128).T
    biasb[:, 6] = np.asarray(inputs["b3"], np.float32)
    biasb[:, 7:9] = np.asarray(inputs["b4"], np.float32).reshape(2, 128).T
    biasb[:, 9:13] = np.asarray(inputs["b5"], np.float32).reshape(4, 128).T
    biasb[:, 13:26] = biasb[:, 0:13] * EW
    biasb[:, 19] = biasb[:, 6]  # L3 vector path uses (ps*(1/64)) + b3 raw

    const_map = {
        "wenc": wenc,
        "biasb": biasb,
        "w6p": w6blob,
    }

    in_maps = []
    for c in range(NCORES):
        sl = slice(c * BC, (c + 1) * BC)
        xc = x_full[sl]  # [BC, 600]
        xw = np.zeros((128, NI4 * 512), np.float32)
        for i in range(NW):
            i4, wpos = divmod(i, 4)
            p0 = 32 * wpos
            ncols = min(30, D_IN - 24 * i)
            xw[p0:p0 + ncols, i4 * 512:i4 * 512 + BC] = xc[:, 24 * i:24 * i + ncols].T
            xw[p0 + 30, i4 * 512:i4 * 512 + BC] = 1.0
        xbg = np.zeros((128, NI4 * 512 + NI4 * WT), BF16_NP)
        xbg[:, :NI4 * 512] = xw.astype(BF16_NP)
        xbg[:, NI4 * 512:] = gmat
        l4emb = np.zeros((32, BC), np.float32)
        l4emb[0:LBL_DIM] = emb[labels[sl]].T
        m = dict(const_map)
        m["x8"] = xw.astype(F8_NP)
        m["xbg"] = xbg
        m["l4emb"] = l4emb.astype(F8_NP)
        in_maps.append(m)
    return in_maps


_NC_CACHE = None


def kernel(**inputs) -> np.ndarray:
    global _NC_CACHE
    if _NC_CACHE is None:
        _NC_CACHE = _build_nc()
    nc = _NC_CACHE
    in_maps = _host_prep(inputs)
    res = bass_utils.run_bass_kernel_spmd(nc, in_maps, core_ids=list(range(NCORES)))
    out = np.concatenate([res.results[c]["y"] for c in range(NCORES)], axis=0)
    return out.astype(np.float32).reshape(B, HIGH_T, FEAT)
